# revision 12
# baseline (speedup 1.0000x reference)
"""Multi-head attention (B=4, S=2048, D=1024, H=16) on 8 NeuronCores.

Sharding: core c handles batch b = c//2 and head-group g = c%2 (8 heads each).
W_q/W_k/W_v are column-split per head group; W_o row-split; each core computes
a partial output for its batch which the host sums (row-parallel reduction).

Device layout strategy (per core):
  - inputs host-pretransposed: qt/kt/vt = X[b].T  [D, S]  (d on partitions)
  - q^T, k^T computed in [o, s] layout (o = head*64+dk on partitions)
  - v computed in natural [s, o] layout
  - scores^T tile [sk, sq] = k_h^T.T @ q_h^T  (K=dk=64; head pairs row-packed)
  - E^T = exp(scores^T/8) via ACT directly PSUM->SBUF
  - num^T[dk, sq] += v_chunk.T @ E^T  (head pairs col-packed into one PSUM bank)
  - softmax denominators via ones-column matmuls (M=1, col-packed)
  - normalize: one DVE multiply per pair with gpsimd-broadcast reciprocals
  - Y = O^T.T @ Wo^T chunks accumulated over o; host adds the two partials + bo

All matmuls run in float32r (full fp32 storage, ~1.5e-4 rel err, 1 cyc/row).
"""

import numpy as np

import concourse.bass as bass
import concourse.tile as tile
from concourse import bacc, mybir
from concourse.bass_utils import run_bass_kernel_spmd

FP = mybir.dt.float32
FR = mybir.dt.float32r
AF = mybir.ActivationFunctionType

B, S, D = 4, 2048, 1024
H, DK = 16, 64
HPC = 8          # heads per core
OC = HPC * DK    # 512 output cols per core
N_CORES = 8

_PROG_CACHE = {}


def r(x):
    return x


def build_program(repeats: int = 1, debug_dump: bool = False):
    """Build + compile the per-core Bass program. `repeats` re-executes the
    whole body in a dynamic loop (for timing amplification only)."""
    nc = bacc.Bacc("TRN2", target_bir_lowering=False, debug=False,
                   num_devices=N_CORES)

    qt = nc.dram_tensor("qt", [D, S], FR, kind="ExternalInput").ap()
    kt = nc.dram_tensor("kt", [D, S], FR, kind="ExternalInput").ap()
    vt = nc.dram_tensor("vt", [D, S], FR, kind="ExternalInput").ap()
    wqt = nc.dram_tensor("wqt", [D, OC], FR, kind="ExternalInput").ap()
    wkt = nc.dram_tensor("wkt", [D, OC], FR, kind="ExternalInput").ap()
    wvt = nc.dram_tensor("wvt", [D, OC], FR, kind="ExternalInput").ap()
    wot = nc.dram_tensor("wot", [OC, D], FR, kind="ExternalInput").ap()
    bq = nc.dram_tensor("bq", [OC], FP, kind="ExternalInput").ap()
    bk = nc.dram_tensor("bk", [OC], FP, kind="ExternalInput").ap()
    bv = nc.dram_tensor("bv", [OC], FP, kind="ExternalInput").ap()
    onec = nc.dram_tensor("onec", [128, HPC], FR, kind="ExternalInput").ap()
    y = nc.dram_tensor("y", [S, D], FP, kind="ExternalOutput").ap()
    if debug_dump:
        dbg_qt = nc.dram_tensor("dbg_qt", [OC, S], FP, kind="ExternalOutput").ap()
        dbg_kt = nc.dram_tensor("dbg_kt", [OC, S], FP, kind="ExternalOutput").ap()
        dbg_v = nc.dram_tensor("dbg_v", [S, HPC * 65], FP, kind="ExternalOutput").ap()
        dbg_ot = nc.dram_tensor("dbg_ot", [OC, S], FP, kind="ExternalOutput").ap()
        dbg_et = nc.dram_tensor("dbg_et", [128, 512], FP, kind="ExternalOutput").ap()
        dbg_num = nc.dram_tensor("dbg_num", [65, 512], FP, kind="ExternalOutput").ap()
        dbg_bc = nc.dram_tensor("dbg_bc", [64, 512], FP, kind="ExternalOutput").ap()
        dbg_rec = nc.dram_tensor("dbg_rec", [1, 512], FP, kind="ExternalOutput").ap()

    ND = D // 128    # 8 d-tiles
    NS = S // 128    # 16 s-tiles
    NSB = S // 512   # 4 s-blocks
    NO = OC // 128   # 4 o-tiles per core

    with tile.TileContext(nc) as tc:
        def body(_iv=None):
            with tc.tile_pool(name="pers_o", bufs=1) as pers_o, \
                 tc.tile_pool(name="consts", bufs=1) as consts, \
                 tc.tile_pool(name="psum", bufs=1, space="PSUM") as psum:
                oT = [pers_o.tile([128, S], FR, tag=f"oT{i}", name=f"oT{i}") for i in range(NO)]
                ones_sb = consts.tile([128, HPC], FR, tag="ones")
                nc.sync.dma_start(out=ones_sb[:], in_=onec[:])
                bq_t = consts.tile([128, NO], FP, tag="bq")
                nc.sync.dma_start(out=bq_t[:], in_=bq.rearrange("(ot oi) -> oi ot", oi=128))
                bk_t = consts.tile([128, NO], FP, tag="bk")
                nc.sync.dma_start(out=bk_t[:], in_=bk.rearrange("(ot oi) -> oi ot", oi=128))
                bv_bc = consts.tile([128, OC], FP, tag="bv")
                nc.sync.dma_start(out=bv_bc[:], in_=bv.partition_broadcast(128))

                with tc.tile_pool(name="pers_qkv", bufs=1) as pers:
                    qT = [pers.tile([128, S], FR, tag=f"qT{i}", name=f"qT{i}") for i in range(NO)]
                    kT = [pers.tile([128, S], FR, tag=f"kT{i}", name=f"kT{i}") for i in range(NO)]
                    vsb = [pers.tile([128, HPC * 65], FR, tag=f"v{i}", name=f"v{i}") for i in range(NS)]

                    # ---------------- projections ----------------
                    with tc.tile_pool(name="stage", bufs=2) as stage, \
                         tc.tile_pool(name="wstage", bufs=1) as wstage:
                        # q^T and k^T: out[o,s] tiles; lhsT = W^T chunk, rhs = X^T chunk
                        for name, xt_d, wt_d, bias_t, outT in (
                            ("q", qt, wqt, bq_t, qT),
                            ("k", kt, wkt, bk_t, kT),
                        ):
                            wt = wstage.tile([128, ND, OC], FR, tag="w")
                            nc.sync.dma_start(
                                out=wt[:],
                                in_=wt_d.rearrange("(dd di) o -> di dd o", di=128))
                            for sb in range(NSB):
                                xs = stage.tile([128, ND, 512], FR, tag="xs")
                                nc.sync.dma_start(
                                    out=xs[:],
                                    in_=xt_d.rearrange("(dd di) s -> di dd s", di=128)
                                        [:, :, sb * 512:(sb + 1) * 512])
                                for ot in range(NO):
                                    ps = psum.tile([128, 512], FP, tag="mm", bufs=2, name="ps")
                                    for dt in range(ND):
                                        nc.tensor.matmul(
                                            ps[:],
                                            r(wt[:, dt, ot * 128:(ot + 1) * 128]),
                                            r(xs[:, dt, :]),
                                            start=(dt == 0), stop=(dt == ND - 1))
                                    nc.vector.tensor_scalar_add(
                                        outT[ot][:, sb * 512:(sb + 1) * 512],
                                        ps[:], bias_t[:, ot:ot + 1])

                        # v natural: out[s,o]; lhsT = V^T chunk, rhs = Wv^T chunk
                        wt = wstage.tile([128, ND, OC], FR, tag="w")
                        nc.sync.dma_start(
                            out=wt[:],
                            in_=wvt.rearrange("(dd di) o -> di dd o", di=128))
                        for st in range(NS):
                            xs = stage.tile([128, ND, 128], FR, tag="xs")
                            nc.sync.dma_start(
                                out=xs[:],
                                in_=vt.rearrange("(dd di) s -> di dd s", di=128)
                                    [:, :, st * 128:(st + 1) * 128])
                            ps = psum.tile([128, 512], FP, tag="mm", bufs=2, name="ps")
                            for dt in range(ND):
                                nc.tensor.matmul(
                                    ps[:], r(xs[:, dt, :]), r(wt[:, dt, :]),
                                    start=(dt == 0), stop=(dt == ND - 1))
                            vv = vsb[st].rearrange("p (h c) -> p h c", c=65)
                            nc.vector.tensor_add(
                                vv[:, :, 0:64],
                                ps.rearrange("p (h c) -> p h c", c=64),
                                bv_bc.rearrange("p (h c) -> p h c", c=64))
                            nc.vector.tensor_copy(vv[:, :, 64:65], ones_sb.unsqueeze(2))

                    if debug_dump:
                        for i in range(NO):
                            nc.sync.dma_start(out=dbg_qt[i * 128:(i + 1) * 128, :], in_=qT[i][:].bitcast(FP))
                            nc.sync.dma_start(out=dbg_kt[i * 128:(i + 1) * 128, :], in_=kT[i][:].bitcast(FP))
                        for i in range(NS):
                            nc.sync.dma_start(out=dbg_v[i * 128:(i + 1) * 128, :], in_=vsb[i][:].bitcast(FP))

                    # ---------------- attention ----------------
                    with tc.tile_pool(name="et", bufs=5) as epool, \
                         tc.tile_pool(name="nrm", bufs=2) as npool:
                        for p in range(NO):          # head pair p -> heads 2p, 2p+1
                            for sq in range(NSB):
                                nump = [psum.tile([65, 512], FP, tag="num", bufs=3,
                                                  name=f"nump{e}") for e in range(2)]
                                for sk in range(NS):
                                    ets = []
                                    for e in range(2):
                                        scs = psum.tile([128, 512], FP, tag="sc", bufs=3, name="scs")
                                        nc.tensor.matmul(
                                            scs[:],
                                            kT[p][e * 64:(e + 1) * 64,
                                                  sk * 128:(sk + 1) * 128],
                                            qT[p][e * 64:(e + 1) * 64,
                                                  sq * 512:(sq + 1) * 512],
                                            start=True, stop=True)
                                        et = epool.tile([128, 512], FR, tag=f"et{e}", name=f"et{e}")
                                        nc.scalar.activation(et[:], scs[:], AF.Exp,
                                                             scale=0.125)
                                        if debug_dump and p == 0 and sq == 0 and sk == 0 and e == 0:
                                            nc.sync.dma_start(out=dbg_et[:], in_=et[:].bitcast(FP))
                                        ets.append(et)
                                    for e in range(2):
                                        h = 2 * p + e
                                        nc.tensor.matmul(
                                            nump[e][:],
                                            vsb[sk][:, h * 65:(h + 1) * 65],
                                            ets[e][:],
                                            start=(sk == 0), stop=(sk == NS - 1),
                                            skip_group_check=True)
                                if debug_dump and p == 0 and sq == 0:
                                    dnt = npool.tile([65, 512], FP, tag="dnt", name="dnt")
                                    nc.scalar.copy(dnt[:], nump[0][:])
                                    nc.sync.dma_start(out=dbg_num[:], in_=dnt[:])
                                for e in range(2):
                                    rec = npool.tile([65, 512], FP, tag="rec", name="rec")
                                    nc.vector.reciprocal(rec[64:65, :], nump[e][64:65, :])
                                    rec0 = npool.tile([1, 512], FP, tag="rec0", name="rec0")
                                    nc.sync.dma_start(out=rec0[:], in_=rec[64:65, :])
                                    bc = npool.tile([64, 512], FP, tag="bc", name="bc")
                                    nc.gpsimd.partition_broadcast(bc[:], rec0[:])
                                    if debug_dump and p == 0 and sq == 0 and e == 0:
                                        nc.sync.dma_start(out=dbg_bc[:], in_=bc[:])
                                        nc.sync.dma_start(out=dbg_rec[:], in_=rec[64:65, :])
                                    if e == 0:
                                        nc.vector.tensor_mul(
                                            oT[p][0:64, sq * 512:(sq + 1) * 512],
                                            nump[e][0:64, :], bc[:])
                                    else:
                                        tmp = npool.tile([64, 512], FR, tag="tmp", name="tmp")
                                        nc.vector.tensor_mul(tmp[:], nump[e][0:64, :], bc[:])
                                        nc.sync.dma_start(
                                            out=oT[p][64:128, sq * 512:(sq + 1) * 512],
                                            in_=tmp[:])

                if debug_dump:
                    for i in range(NO):
                        nc.sync.dma_start(out=dbg_ot[i * 128:(i + 1) * 128, :], in_=oT[i][:].bitcast(FP))

                # ---------------- output projection ----------------
                with tc.tile_pool(name="fstage", bufs=1) as fstage, \
                     tc.tile_pool(name="yout", bufs=3) as ypool:
                    wo_t = fstage.tile([128, NO, D], FR, tag="wo")
                    nc.sync.dma_start(
                        out=wo_t[:],
                        in_=wot.rearrange("(oo oi) yd -> oi oo yd", oi=128))
                    for st in range(NS):
                        for yb in range(2):
                            ps = psum.tile([128, 512], FP, tag="mm", bufs=2, name="ps")
                            for o4 in range(NO):
                                nc.tensor.matmul(
                                    ps[:],
                                    r(oT[o4][:, st * 128:(st + 1) * 128]),
                                    r(wo_t[:, o4, yb * 512:(yb + 1) * 512]),
                                    start=(o4 == 0), stop=(o4 == NO - 1))
                            yt = ypool.tile([128, 512], FP, tag="yt")
                            nc.scalar.copy(yt[:], ps[:])
                            nc.sync.dma_start(
                                out=y[st * 128:(st + 1) * 128,
                                      yb * 512:(yb + 1) * 512],
                                in_=yt[:])

        if repeats == 1:
            body()
        else:
            with tc.For_i(0, repeats, 1) as iv:
                body(iv)

    nc.compile()
    return nc


def _get_prog(repeats: int = 1):
    if repeats not in _PROG_CACHE:
        _PROG_CACHE[repeats] = build_program(repeats)
    return _PROG_CACHE[repeats]


def make_in_maps(Q, K, V, Wq, bq, Wk, bk, Wv, bv, Wo, bo):
    Q, K, V = (np.asarray(x, dtype=np.float32) for x in (Q, K, V))
    Wq, Wk, Wv, Wo = (np.asarray(x, dtype=np.float32) for x in (Wq, Wk, Wv, Wo))
    bq, bk, bv = (np.asarray(x, dtype=np.float32) for x in (bq, bk, bv))

    qt_b = [np.ascontiguousarray(Q[b].T) for b in range(B)]
    kt_b = [np.ascontiguousarray(K[b].T) for b in range(B)]
    vt_b = [np.ascontiguousarray(V[b].T) for b in range(B)]
    # per head-group g: out-col slice of the projections
    wqt_g = [np.ascontiguousarray(Wq.T[:, g * OC:(g + 1) * OC]) for g in range(2)]
    wkt_g = [np.ascontiguousarray(Wk.T[:, g * OC:(g + 1) * OC]) for g in range(2)]
    wvt_g = [np.ascontiguousarray(Wv.T[:, g * OC:(g + 1) * OC]) for g in range(2)]
    wot_g = [np.ascontiguousarray(Wo.T[g * OC:(g + 1) * OC, :]) for g in range(2)]

    in_maps = []
    for c in range(N_CORES):
        b, g = c // 2, c % 2
        in_maps.append({
            "qt": qt_b[b], "kt": kt_b[b], "vt": vt_b[b],
            "wqt": wqt_g[g], "wkt": wkt_g[g], "wvt": wvt_g[g],
            "wot": wot_g[g],
            "bq": np.ascontiguousarray(bq[g * OC:(g + 1) * OC]),
            "bk": np.ascontiguousarray(bk[g * OC:(g + 1) * OC]),
            "bv": np.ascontiguousarray(bv[g * OC:(g + 1) * OC]),
            "onec": np.ones((128, HPC), dtype=np.float32),
        })
    return in_maps


def gather_output(results, bo):
    bo = np.asarray(bo, dtype=np.float32)
    Y = np.empty((B, S, D), dtype=np.float32)
    for b in range(B):
        Y[b] = results[2 * b]["y"] + results[2 * b + 1]["y"] + bo
    return Y


def kernel(Q, K, V, Wq, bq, Wk, bk, Wv, bv, Wo, bo):
    nc = _get_prog()
    in_maps = make_in_maps(Q, K, V, Wq, bq, Wk, bk, Wv, bv, Wo, bo)
    res = run_bass_kernel_spmd(nc, in_maps, list(range(N_CORES)))
    return gather_output(res.results, bo)
